# revision 14
# baseline (speedup 1.0000x reference)
"""Pointer-generator decoder step on 8 Trainium2 NeuronCores.

Batch-parallel over B=512 (64 rows per core). Inside each core:
  - context  = einsum(attn, enc_out)      -> DVE tensor_tensor_reduce over [e,s] tiles
  - embed    = emb_table[dec_input]       -> indirect DMA gather
  - LSTM cell (transposed layout, fp32 PE matmuls)
  - coverage attention: wh_app via PE bf16 matmuls (enc transposed), cov*awc folded
    in as a K=1 matmul row, ws_app folded in as the ACT tanh bias, energy via PE,
    softmax via ACT Exp with accumulated row sums
  - p_vocab softmax via PE bf16 matmuls against v_W^T + ACT Exp(accum)
  - final mix + scatter of attention into vocab positions via indirect DMA
    (descriptor order = s ascending -> last-write-wins, matching jax CPU scatter)
"""

import numpy as np
import ml_dtypes

import concourse.bacc as bacc
import concourse.bass as bass
import concourse.mybir as mybir
import concourse.tile as tile
from concourse.bass import IndirectOffsetOnAxis
from concourse.masks import make_identity

BF16 = ml_dtypes.bfloat16
F32 = mybir.dt.float32
BF = mybir.dt.bfloat16
I32 = mybir.dt.int32

B, S, V, E, H, EMB = 512, 400, 50000, 512, 256, 128
NCORES = 8
BC = B // NCORES            # 64 batch rows per core
NBCH = 4                    # batch chunks per core
BCH = BC // NBCH            # 16 rows per chunk
HBCH = BCH // 2             # 8 rows per half-chunk (slab granularity)
ET = E // 128               # 4 e-tiles
HT = H // 128               # 2 h-tiles
KX = (E + EMB) // 128       # 5 k-tiles of x = [context, embed]
NMT = (4 * H) // 128        # 8 gate m-tiles
VSL = 5000                  # v_W slab width
NVSL = V // VSL             # 10 slabs
VCH = 500                   # logits psum chunk width
NVCH = VSL // VCH           # 10 chunks per slab

AF = mybir.ActivationFunctionType
OP = mybir.AluOpType

_CACHE = {}


def _emit(nc, tc):
    # ---------------- DRAM tensors ----------------
    d_encT = nc.dram_tensor("encT", [ET, 128, BC, S], BF, kind="ExternalInput")
    d_arep = nc.dram_tensor("arep", [128, BC, S], BF, kind="ExternalInput")
    # coverage split hi/lo bf16 so the cov*awc fold stays ~fp32-accurate
    d_covbf = nc.dram_tensor("covbf", [2, BC * S], BF, kind="ExternalInput")
    d_cov = nc.dram_tensor("cov", [BC, S], F32, kind="ExternalInput")
    d_h0T = nc.dram_tensor("h0T", [H, BC], F32, kind="ExternalInput")
    d_c0T = nc.dram_tensor("c0T", [H, BC], F32, kind="ExternalInput")
    d_dec = nc.dram_tensor("dec", [BC, 1], I32, kind="ExternalInput")
    d_emb = nc.dram_tensor("emb", [V, EMB], F32, kind="ExternalInput")
    d_scat = nc.dram_tensor("scat", [BC, S], I32, kind="ExternalInput")
    d_WihT = nc.dram_tensor("WihT", [E + EMB, 4 * H], F32, kind="ExternalInput")
    d_WhhT = nc.dram_tensor("WhhT", [H, 4 * H], F32, kind="ExternalInput")
    d_bihT = nc.dram_tensor("bihT", [128, NMT], F32, kind="ExternalInput")
    d_bhhT = nc.dram_tensor("bhhT", [128, NMT], F32, kind="ExternalInput")
    d_awhWT = nc.dram_tensor("awhWT", [ET, 128, HT, 128], BF, kind="ExternalInput")
    d_awhbT = nc.dram_tensor("awhbT", [128, HT], F32, kind="ExternalInput")
    d_awsbT = nc.dram_tensor("awsbT", [128, HT], F32, kind="ExternalInput")
    d_awsWT = nc.dram_tensor("awsWT", [HT, 128, HT, 128], F32, kind="ExternalInput")
    d_awc = nc.dram_tensor("awc", [1, H], BF, kind="ExternalInput")
    d_avT = nc.dram_tensor("avT", [128, HT], F32, kind="ExternalInput")
    d_whT = nc.dram_tensor("whT", [128, 4], F32, kind="ExternalInput")
    d_wxT = nc.dram_tensor("wxT", [128, 1], F32, kind="ExternalInput")
    d_wsT = nc.dram_tensor("wsT", [128, 2], F32, kind="ExternalInput")
    d_vWT = nc.dram_tensor("vWT", [H, V], BF, kind="ExternalInput")

    d_out = nc.dram_tensor("out", [BC, V], F32, kind="ExternalOutput")
    d_ncov = nc.dram_tensor("ncov", [BC, S], F32, kind="ExternalOutput")
    d_nattn = nc.dram_tensor("nattn", [BC, S], F32, kind="ExternalOutput")
    d_htT = nc.dram_tensor("htT", [H, BC], F32, kind="ExternalOutput")
    d_ctT = nc.dram_tensor("ctT", [H, BC], F32, kind="ExternalOutput")
    d_loss = nc.dram_tensor("loss", [1, 1], F32, kind="ExternalOutput")

    with tc.tile_pool(name="const", bufs=1) as cp:
        ident = cp.tile([128, 128], F32)
        make_identity(nc, ident[:])
        ones_col = cp.tile([BC, 1], F32)
        nc.vector.memset(ones_col[:], 1.0)

        h0T = cp.tile([128, HT, BC], F32)
        nc.sync.dma_start(out=h0T[:], in_=d_h0T.ap().rearrange("(t p) b -> p t b", p=128))
        c0T = cp.tile([128, HT, BC], F32)
        nc.sync.dma_start(out=c0T[:], in_=d_c0T.ap().rearrange("(t p) b -> p t b", p=128))

        bihT = cp.tile([128, NMT], F32)
        nc.sync.dma_start(out=bihT[:], in_=d_bihT.ap())
        bhhT = cp.tile([128, NMT], F32)
        nc.sync.dma_start(out=bhhT[:], in_=d_bhhT.ap())
        bias_g = cp.tile([128, NMT], F32)
        nc.vector.tensor_tensor(out=bias_g[:], in0=bihT[:], in1=bhhT[:], op=OP.add)
        nbias_g = cp.tile([128, NMT], F32)
        nc.vector.tensor_scalar(out=nbias_g[:], in0=bias_g[:],
                                scalar1=-1.0, scalar2=None, op0=OP.mult)

        awhbT = cp.tile([128, HT], F32)
        nc.sync.dma_start(out=awhbT[:], in_=d_awhbT.ap())
        awsbT = cp.tile([128, HT], F32)
        nc.sync.dma_start(out=awsbT[:], in_=d_awsbT.ap())
        bias_wa = cp.tile([128, HT], F32)
        nc.vector.tensor_tensor(out=bias_wa[:], in0=awhbT[:], in1=awsbT[:], op=OP.add)

        awc = cp.tile([1, H], BF)
        nc.sync.dma_start(out=awc[:], in_=d_awc.ap())
        avF = cp.tile([128, HT], F32)
        nc.sync.dma_start(out=avF[:], in_=d_avT.ap())
        whT = cp.tile([128, 4], F32)
        nc.sync.dma_start(out=whT[:], in_=d_whT.ap())
        wxT = cp.tile([128, 1], F32)
        nc.sync.dma_start(out=wxT[:], in_=d_wxT.ap())
        wsT = cp.tile([128, 2], F32)
        nc.sync.dma_start(out=wsT[:], in_=d_wsT.ap())

        dec_sb = cp.tile([BC, 1], I32)
        nc.sync.dma_start(out=dec_sb[:], in_=d_dec.ap())
        emb_sb = cp.tile([BC, EMB], F32)
        nc.gpsimd.indirect_dma_start(
            out=emb_sb[:], out_offset=None,
            in_=d_emb.ap(),
            in_offset=IndirectOffsetOnAxis(ap=dec_sb[:, :1], axis=0),
            bounds_check=V - 1,
        )

        # persistent state, transposed layouts [feature, batch]
        embT = cp.tile([128, BC], F32)
        ctxT = cp.tile([128, ET, BC], F32)
        htT = cp.tile([128, HT, BC], F32)
        ctT = cp.tile([128, HT, BC], F32)
        wsappT = cp.tile([128, HT, BC], F32)
        sums_row = cp.tile([1, BC], F32)
        pgen_row = cp.tile([1, BC], F32)

        # ============ phase 1: context + LSTM + attention ============
        with tc.tile_pool(name="wp", bufs=1) as wp, \
             tc.tile_pool(name="ch", bufs=2) as chp, \
             tc.tile_pool(name="ps1", bufs=2, space="PSUM") as pp:

            WihT = wp.tile([128, KX, NMT, 128], F32)
            nc.sync.dma_start(
                out=WihT[:],
                in_=d_WihT.ap().rearrange("(kt p) (mt m) -> p kt mt m", p=128, m=128))
            WhhT = wp.tile([128, HT, NMT, 128], F32)
            nc.sync.dma_start(
                out=WhhT[:],
                in_=d_WhhT.ap().rearrange("(kt p) (mt m) -> p kt mt m", p=128, m=128))
            awsWT = wp.tile([128, HT, HT, 128], F32)
            nc.sync.dma_start(
                out=awsWT[:],
                in_=d_awsWT.ap().rearrange("kt p mt m -> p kt mt m"))
            awhWT = wp.tile([128, ET, HT, 128], BF)
            nc.sync.dma_start(
                out=awhWT[:],
                in_=d_awhWT.ap().rearrange("t p h m -> p t h m"))

            # embed transpose -> embT
            ps_e = pp.tile([128, BC], F32, tag="small")
            nc.tensor.transpose(out=ps_e[:], in_=emb_sb[:], identity=ident[:BC, :BC])
            nc.vector.tensor_copy(out=embT[:], in_=ps_e[:])

            for ch in range(NBCH):
                b0 = ch * BCH
                cs = slice(b0, b0 + BCH)
                # -------- load half-chunk slabs --------
                EH = []   # EH[half] = [128, ET, HBCH*S]
                AH = []
                CHI = []
                CLO = []
                XH = []
                for hf in range(2):
                    hb = b0 + hf * HBCH
                    e_ = chp.tile([128, ET, HBCH * S], BF, tag="eall")
                    nc.sync.dma_start(
                        out=e_[:],
                        in_=d_encT.ap()[:, :, hb:hb + HBCH, :]
                            .rearrange("t p b s -> p t (b s)"))
                    EH.append(e_)
                    a_ = chp.tile([128, HBCH * S], BF, tag="arep")
                    nc.sync.dma_start(
                        out=a_[:],
                        in_=d_arep.ap()[:, hb:hb + HBCH, :].rearrange("p b s -> p (b s)"))
                    AH.append(a_)
                    chi = chp.tile([1, HBCH * S], BF, tag="covhi")
                    nc.sync.dma_start(
                        out=chi[:], in_=d_covbf.ap()[0:1, hb * S:(hb + HBCH) * S])
                    CHI.append(chi)
                    clo = chp.tile([1, HBCH * S], BF, tag="covlo")
                    nc.sync.dma_start(
                        out=clo[:], in_=d_covbf.ap()[1:2, hb * S:(hb + HBCH) * S])
                    CLO.append(clo)
                    x_ = chp.tile([1, HBCH * S], F32, tag="exprow",
                                  name=f"exprow_{ch}_{hf}")
                    XH.append(x_)

                # -------- context via DVE multiply+reduce --------
                for bl in range(BCH):
                    hf, bh = divmod(bl, HBCH)
                    Bg = b0 + bl
                    sl = slice(bh * S, (bh + 1) * S)
                    for t in range(ET):
                        scr = chp.tile([128, S], BF, tag="ttrscr")
                        nc.vector.tensor_tensor_reduce(
                            out=scr[:], in0=EH[hf][:, t, sl], in1=AH[hf][:, sl],
                            scale=1.0, scalar=0.0, op0=OP.mult, op1=OP.add,
                            accum_out=ctxT[:, t, Bg:Bg + 1])

                # -------- LSTM gates (transposed, fp32) --------
                xk = [ctxT[:, t, cs] for t in range(ET)] + [embT[:, cs]]
                gts = chp.tile([128, NMT, BCH], F32, tag="gates")
                for mt in range(NMT):
                    pg = pp.tile([128, BCH], F32, tag="small")
                    for kt in range(KX):
                        nc.tensor.matmul(pg[:], lhsT=WihT[:, kt, mt, :], rhs=xk[kt],
                                         start=(kt == 0), stop=False)
                    for kt in range(HT):
                        nc.tensor.matmul(pg[:], lhsT=WhhT[:, kt, mt, :],
                                         rhs=h0T[:, kt, cs],
                                         start=False, stop=(kt == HT - 1))
                    if mt in (4, 5):
                        nc.scalar.activation(out=gts[:, mt, :], in_=pg[:], func=AF.Tanh,
                                             bias=bias_g[:, mt:mt + 1])
                    else:
                        # sigmoid via exp to keep ACT on one table set:
                        # sigmoid(x+b) = 1 / (1 + e^(-x-b))
                        eg = chp.tile([128, BCH], F32, tag="eg")
                        nc.scalar.activation(out=eg[:], in_=pg[:], func=AF.Exp,
                                             scale=-1.0, bias=nbias_g[:, mt:mt + 1])
                        nc.vector.tensor_scalar(out=eg[:], in0=eg[:],
                                                scalar1=1.0, scalar2=None, op0=OP.add)
                        nc.vector.reciprocal(out=gts[:, mt, :], in_=eg[:])
                for ht in range(HT):
                    i_, f_ = gts[:, 0 + ht, :], gts[:, 2 + ht, :]
                    g_, o_ = gts[:, 4 + ht, :], gts[:, 6 + ht, :]
                    t1 = chp.tile([128, BCH], F32, tag="t1")
                    nc.vector.tensor_tensor(out=t1[:], in0=f_, in1=c0T[:, ht, cs], op=OP.mult)
                    t2 = chp.tile([128, BCH], F32, tag="t2")
                    nc.vector.tensor_tensor(out=t2[:], in0=i_, in1=g_, op=OP.mult)
                    nc.vector.tensor_tensor(out=ctT[:, ht, cs], in0=t1[:], in1=t2[:], op=OP.add)
                    t3 = chp.tile([128, BCH], F32, tag="t3")
                    nc.scalar.activation(out=t3[:], in_=ctT[:, ht, cs], func=AF.Tanh)
                    nc.vector.tensor_tensor(out=htT[:, ht, cs], in0=o_, in1=t3[:], op=OP.mult)

                # -------- ws_app (transposed) --------
                for mt in range(HT):
                    pw = pp.tile([128, BCH], F32, tag="small")
                    for kt in range(HT):
                        nc.tensor.matmul(pw[:], lhsT=awsWT[:, kt, mt, :],
                                         rhs=htT[:, kt, cs],
                                         start=(kt == 0), stop=(kt == HT - 1))
                    nc.scalar.activation(out=wsappT[:, mt, cs], in_=pw[:],
                                         func=AF.Identity, bias=bias_wa[:, mt:mt + 1])

                # -------- p_gen row --------
                ppg = pp.tile([1, BCH], F32, tag="small")
                mms = [(whT[:, j:j + 1], ctxT[:, j, cs]) for j in range(ET)]
                mms.append((wxT[:, 0:1], embT[:, cs]))
                mms += [(wsT[:, j:j + 1], htT[:, j, cs]) for j in range(HT)]
                for j, (lh, rh) in enumerate(mms):
                    nc.tensor.matmul(ppg[:], lhsT=lh, rhs=rh,
                                     start=(j == 0), stop=(j == len(mms) - 1))
                epg = chp.tile([1, BCH], F32, tag="epg")
                nc.scalar.activation(out=epg[:], in_=ppg[:], func=AF.Exp, scale=-1.0)
                nc.vector.tensor_scalar(out=epg[:], in0=epg[:],
                                        scalar1=1.0, scalar2=None, op0=OP.add)
                nc.vector.reciprocal(out=pgen_row[:, cs], in_=epg[:])

                # -------- coverage attention per row --------
                for bl in range(BCH):
                    hf, bh = divmod(bl, HBCH)
                    Bg = b0 + bl
                    sl = slice(bh * S, (bh + 1) * S)
                    tnh = chp.tile([128, HT, S], F32, tag="tanhb")
                    for ht in range(HT):
                        pw = pp.tile([128, S], F32, tag=f"whapp{ht}")
                        for t in range(ET):
                            nc.tensor.matmul(pw[:], lhsT=awhWT[:, t, ht, :],
                                             rhs=EH[hf][:, t, sl],
                                             start=(t == 0), stop=False)
                        nc.tensor.matmul(pw[:], lhsT=awc[:, ht * 128:(ht + 1) * 128],
                                         rhs=CHI[hf][:, sl], start=False, stop=False)
                        nc.tensor.matmul(pw[:], lhsT=awc[:, ht * 128:(ht + 1) * 128],
                                         rhs=CLO[hf][:, sl], start=False, stop=True)
                        nc.scalar.activation(out=tnh[:, ht, :], in_=pw[:], func=AF.Tanh,
                                             bias=wsappT[:, ht, Bg:Bg + 1])
                    pe_ = pp.tile([1, S], F32, tag="energy")
                    for ht in range(HT):
                        nc.tensor.matmul(pe_[:],
                                         lhsT=avF[:, ht:ht + 1].bitcast(mybir.dt.float32r),
                                         rhs=tnh[:, ht, :].bitcast(mybir.dt.float32r),
                                         start=(ht == 0), stop=(ht == HT - 1))
                    nc.scalar.activation(out=XH[hf][:, bh * S:(bh + 1) * S], in_=pe_[:],
                                         func=AF.Exp,
                                         accum_out=sums_row[:, Bg:Bg + 1])
                # exp rows -> DRAM (flat view of nattn)
                flat_na = d_nattn.ap().rearrange("b s -> (b s)")[None, :]
                for hf in range(2):
                    hb = b0 + hf * HBCH
                    nc.sync.dma_start(out=flat_na[:, hb * S:(hb + HBCH) * S], in_=XH[hf][:])

        # ============ phase 2: softmax finish, vocab, scatter ============
        with tc.tile_pool(name="p2", bufs=1) as p2, \
             tc.tile_pool(name="vw", bufs=2) as vwp, \
             tc.tile_pool(name="ps2", bufs=4, space="PSUM") as pp2:

            # read exp rows back batch-major
            attn_e = p2.tile([BC, S], F32)
            nc.sync.dma_start(out=attn_e[:], in_=d_nattn.ap())

            # transposes [1,BC] -> [BC,1]
            ps_t = pp2.tile([BC, 1], F32, tag="small")
            nc.tensor.transpose(out=ps_t[:], in_=sums_row[:], identity=ident[:1, :1])
            sums_col = p2.tile([BC, 1], F32)
            nc.vector.tensor_copy(out=sums_col[:], in_=ps_t[:])
            ps_t2 = pp2.tile([BC, 1], F32, tag="small")
            nc.tensor.transpose(out=ps_t2[:], in_=pgen_row[:], identity=ident[:1, :1])
            pgen_col = p2.tile([BC, 1], F32)
            nc.vector.tensor_copy(out=pgen_col[:], in_=ps_t2[:])

            recip_a = p2.tile([BC, 1], F32)
            nc.vector.reciprocal(out=recip_a[:], in_=sums_col[:])
            nattn_sb = p2.tile([BC, S], F32)
            nc.vector.tensor_scalar(out=nattn_sb[:], in0=attn_e[:],
                                    scalar1=recip_a[:, 0:1], scalar2=None, op0=OP.mult)
            nc.sync.dma_start(out=d_nattn.ap(), in_=nattn_sb[:])

            cov_sb = p2.tile([BC, S], F32)
            nc.sync.dma_start(out=cov_sb[:], in_=d_cov.ap())
            ncov_sb = p2.tile([BC, S], F32)
            nc.vector.tensor_tensor(out=ncov_sb[:], in0=cov_sb[:], in1=nattn_sb[:], op=OP.add)
            nc.sync.dma_start(out=d_ncov.ap(), in_=ncov_sb[:])

            # coverage loss = sum(min(new_attn, coverage))
            scr2 = p2.tile([BC, S], F32)
            loss_col = p2.tile([BC, 1], F32)
            nc.vector.tensor_tensor_reduce(
                out=scr2[:], in0=nattn_sb[:], in1=cov_sb[:], scale=1.0, scalar=0.0,
                op0=OP.min, op1=OP.add, accum_out=loss_col[:])
            ps_l = pp2.tile([1, 1], F32, tag="small")
            nc.tensor.matmul(ps_l[:], lhsT=loss_col[:], rhs=ones_col[:], start=True, stop=True)
            loss_sb = p2.tile([1, 1], F32)
            nc.vector.tensor_copy(out=loss_sb[:], in_=ps_l[:])
            nc.sync.dma_start(out=d_loss.ap(), in_=loss_sb[:])

            # h_t / c_t out
            for ht in range(HT):
                nc.sync.dma_start(out=d_htT.ap()[ht * 128:(ht + 1) * 128, :], in_=htT[:, ht, :])
                nc.sync.dma_start(out=d_ctT.ap()[ht * 128:(ht + 1) * 128, :], in_=ctT[:, ht, :])

            # -------- vocab distribution --------
            htT_bf = p2.tile([128, HT, BC], BF)
            nc.vector.tensor_copy(out=htT_bf[:], in_=htT[:])
            exp_sb = p2.tile([BC, V], BF)
            sums_ch = p2.tile([BC, NVSL * NVCH], F32)
            for sl_i in range(NVSL):
                vws = []
                for ht in range(HT):
                    vw = vwp.tile([128, VSL], BF, tag=f"vw{ht}")
                    nc.sync.dma_start(
                        out=vw[:],
                        in_=d_vWT.ap()[ht * 128:(ht + 1) * 128, sl_i * VSL:(sl_i + 1) * VSL])
                    vws.append(vw)
                for q in range(NVCH):
                    vo = sl_i * VSL + q * VCH
                    pl = pp2.tile([BC, VCH], F32, tag="logits")
                    for ht in range(HT):
                        nc.tensor.matmul(pl[:], lhsT=htT_bf[:, ht, :],
                                         rhs=vws[ht][:, q * VCH:(q + 1) * VCH],
                                         start=(ht == 0), stop=(ht == HT - 1))
                    nc.scalar.activation(out=exp_sb[:, vo:vo + VCH], in_=pl[:], func=AF.Exp,
                                         accum_out=sums_ch[:, sl_i * NVCH + q:sl_i * NVCH + q + 1])

            sum_v = p2.tile([BC, 1], F32)
            nc.vector.tensor_reduce(out=sum_v[:], in_=sums_ch[:],
                                    axis=mybir.AxisListType.X, op=OP.add)
            recip_v = p2.tile([BC, 1], F32)
            nc.vector.reciprocal(out=recip_v[:], in_=sum_v[:])
            pscale = p2.tile([BC, 1], F32)
            nc.vector.tensor_tensor(out=pscale[:], in0=pgen_col[:], in1=recip_v[:], op=OP.mult)

            NP4 = V // 4
            for part in range(4):
                slv = slice(part * NP4, (part + 1) * NP4)
                nc.vector.tensor_scalar(out=exp_sb[:, slv], in0=exp_sb[:, slv],
                                        scalar1=pscale[:, 0:1], scalar2=None, op0=OP.mult)
                nc.gpsimd.dma_start(out=d_out.ap()[:, slv], in_=exp_sb[:, slv])

            # -------- gather / combine / scatter --------
            scat_sb = p2.tile([BC, S], I32)
            nc.sync.dma_start(out=scat_sb[:], in_=d_scat.ap())
            g_sb = p2.tile([BC, S], F32)
            nc.gpsimd.indirect_dma_start(
                out=g_sb[:], out_offset=None,
                in_=d_out.ap(),
                in_offset=IndirectOffsetOnAxis(ap=scat_sb[:, :], axis=1),
                bounds_check=BC * V - 1,
            )
            omp_col = p2.tile([BC, 1], F32)
            nc.vector.tensor_scalar(out=omp_col[:], in0=pgen_col[:],
                                    scalar1=-1.0, scalar2=1.0, op0=OP.mult, op1=OP.add)
            val_sb = p2.tile([BC, S], F32)
            nc.vector.tensor_scalar(out=val_sb[:], in0=nattn_sb[:],
                                    scalar1=omp_col[:, 0:1], scalar2=None, op0=OP.mult)
            nc.vector.tensor_tensor(out=val_sb[:], in0=val_sb[:], in1=g_sb[:], op=OP.add)
            nc.gpsimd.indirect_dma_start(
                out=d_out.ap(),
                out_offset=IndirectOffsetOnAxis(ap=scat_sb[:, :], axis=1),
                in_=val_sb[:], in_offset=None,
                bounds_check=BC * V - 1,
            )


def build():
    if "nc" in _CACHE:
        return _CACHE["nc"]
    nc = bacc.Bacc("TRN2", target_bir_lowering=False, debug=False, num_devices=NCORES)
    with tile.TileContext(nc) as tc:
        _emit(nc, tc)
    nc.compile()
    _CACHE["nc"] = nc
    return nc


def _prep_core(inp, c):
    b0, b1 = c * BC, (c + 1) * BC
    enc = inp["enc_out"][b0:b1]                       # [BC, S, E] f32
    encT = np.ascontiguousarray(
        enc.astype(BF16).transpose(2, 0, 1)).reshape(ET, 128, BC, S)
    attn = inp["attn"][b0:b1]
    arep = np.ascontiguousarray(
        np.broadcast_to(attn.astype(BF16)[None], (128, BC, S)))
    cov = np.ascontiguousarray(inp["coverage"][b0:b1])
    cov_hi = cov.astype(BF16)
    cov_lo = (cov - cov_hi.astype(np.float32)).astype(BF16)
    m = {
        "encT": encT,
        "arep": arep,
        "cov": cov,
        "covbf": np.stack([cov_hi, cov_lo]).reshape(2, BC * S),
        "h0T": np.ascontiguousarray(inp["h0"][b0:b1].T),
        "c0T": np.ascontiguousarray(inp["c0"][b0:b1].T),
        "dec": inp["dec_input"][b0:b1].astype(np.int32),
        "emb": inp["emb_table"],
        "scat": (np.arange(BC, dtype=np.int32)[:, None] * V
                 + inp["enc_inputs"][b0:b1].astype(np.int32)),
        "WihT": inp["WihT"], "WhhT": inp["WhhT"],
        "bihT": inp["bihT"], "bhhT": inp["bhhT"],
        "awhWT": inp["awhWT"], "awhbT": inp["awhbT"], "awsbT": inp["awsbT"],
        "awsWT": inp["awsWT"], "awc": inp["awc_r"], "avT": inp["avT_r"],
        "whT": inp["whT"], "wxT": inp["wxT"], "wsT": inp["wsT"],
        "vWT": inp["vWT"],
    }
    return m


def _prep_shared(inputs):
    inp = {k: np.asarray(v) for k, v in inputs.items()}
    assert not np.any(inp["v_b"]), "kernel assumes v_b == 0"
    inp["WihT"] = np.ascontiguousarray(inp["W_ih"].T)          # [640, 1024]
    inp["WhhT"] = np.ascontiguousarray(inp["W_hh"].T)          # [256, 1024]
    inp["bihT"] = np.ascontiguousarray(inp["b_ih"].reshape(NMT, 128).T)
    inp["bhhT"] = np.ascontiguousarray(inp["b_hh"].reshape(NMT, 128).T)
    inp["awhWT"] = np.ascontiguousarray(
        inp["awh_W"].T.astype(BF16)).reshape(ET, 128, HT, 128)
    inp["awhbT"] = np.ascontiguousarray(inp["awh_b"].reshape(HT, 128).T)
    inp["awsbT"] = np.ascontiguousarray(inp["aws_b"].reshape(HT, 128).T)
    inp["awsWT"] = np.ascontiguousarray(inp["aws_W"].T).reshape(HT, 128, HT, 128)
    inp["awc_r"] = inp["awc"].astype(BF16).reshape(1, H)
    inp["avT_r"] = np.ascontiguousarray(inp["av"].reshape(HT, 128).T)
    inp["whT"] = np.ascontiguousarray(inp["wh"].reshape(4, 128).T)
    inp["wxT"] = np.ascontiguousarray(inp["wx"].reshape(1, 128).T)
    inp["wsT"] = np.ascontiguousarray(inp["ws"].reshape(2, 128).T)
    inp["vWT"] = np.ascontiguousarray(inp["v_W"].T.astype(BF16))   # [256, V]
    return inp


def kernel(**inputs):
    from concourse.bass_utils import run_bass_kernel_spmd

    inp = _prep_shared(inputs)
    nc = build()
    in_maps = [_prep_core(inp, c) for c in range(NCORES)]
    res = run_bass_kernel_spmd(nc, in_maps, core_ids=list(range(NCORES)))

    out = np.concatenate([r["out"] for r in res.results], axis=0)
    ncov = np.concatenate([r["ncov"] for r in res.results], axis=0)
    nattn = np.concatenate([r["nattn"] for r in res.results], axis=0)
    h_t = np.concatenate([r["htT"].T for r in res.results], axis=0)
    c_t = np.concatenate([r["ctT"].T for r in res.results], axis=0)
    loss = np.float32(sum(float(r["loss"][0, 0]) for r in res.results))
    return out, ncov, (h_t, c_t), nattn, loss
